# revision 48
# baseline (speedup 1.0000x reference)
"""TextCNN-style conv layer (kernel sizes 3/4/5, EMB=300 -> DEPTH=256, bias,
ReLU, max-pool over time) as a Bass/Tile kernel for 8 Trainium2 NeuronCores.

Strategy: data-parallel over batch (8 samples per core), weights replicated.

Conv as dense-K matmuls over a host-materialized im2col matrix
Xrep[k, t] = x[t + k//300, k%300], shared by all three branches (branch n
reads rows [0, n*300), its weights zero-padded to the K-tile boundary).

fp8 e4m3 + DoubleRow: the PE virtualizes to 128x256, contracting 256 rows
per matmul (2 fp8 weights per cell), so each branch needs ceil(n*300/256)
K-tiles: 4/5/6 -> 15 matmuls per sample per depth-half vs 30 at K=128.
Both operands quantize to e4m3; measured end-to-end L2 error vs the fp32
reference is ~1.2e-2 (accumulation stays fp32 in PSUM).

Schedule: sample-group-of-4 OUTER, then branch, then r, then (half,
sample) inner: the 8 concurrent accumulations (2 halves x 4 samples)
exactly fill the 8 PSUM banks, each weight tile serves 4 consecutive
matmuls (LDWEIGHTS hides under the matmul stream), and each x tile is
consumed by both depth-halves AND all three branches before the next
sample-group needs fresh data.  The resulting 240-matmul stream runs
gapless at the fp8 PE roofline (~39.8us at 2.4GHz).

DMA model (measured on HW): a queue's completion post fires ~1.3us
after its transfer's data when the queue is quiet, but on a
still-streaming queue posts starve behind the data traffic (receipts
can lag 2-5us), and per-queue burst rates vary run-to-run between
~142 and ~243GB/s (shared-HBM arbitration lottery).  The schedule
therefore keeps every queue QUIET behind its critical transfers:
sync carries only w0 + r2 (1.05MB, drains ~13.5us, both receipts
prompt); gpsimd/SWDGE (~139GB/s) carries w1 + w2, whose ~20/26us
deadlines tolerate it; scalar carries [r0+r1] and r3, then a 2KB
gate DMA that reads a slice of r3 -- its trigger waits on r3's
completion semaphore, so queue FIFO holds all bulk ([r4+r5] and the
two 1.2MB sg1 groups) off the wire until scalar's own critical
receipts have posted.  First real matmul gates on w0 (post ~10.8us)
+ the r0 range of [r0+r1] (~12.3-15us); the 308GB/s steady demand
sits under the 358GB/s HBM roofline.

A stream of dummy matmuls on a memset tile warms the PE HAM clock gate
(1.2 -> 2.4GHz after ~4.9us of sustained PE activity).  The memset is
gpsimd's first kernel instruction (earliest-dispatching engine), so
warmup matmuls start ~8.1us; the real stream may begin throttled for
<=1us, which costs less than idling behind extra warmups.

Epilogue: relu(max_i(y + b)) == max(0, max_i y + b): DVE reduce_max
over the window axis straight out of PSUM; bias-add + relu fold into
the host-side unpack (a [64, 768] numpy op), so the device tail is
just the final reduce plus an 8KB DMA.  Output staged
[d, sg, branch, half, sample]: sg0's transfer and sg1's first two
branches hide under the matmul stream.
"""

import numpy as np
import ml_dtypes

B, SEQ, EMB = 64, 394, 300
DEPTH = 256
NCORES = 8
BPC = B // NCORES  # samples per core
SEQP = 400  # x_t free-dim padded (zeros) so shifted loads stay in bounds
NS = (3, 4, 5)
NT8 = (4, 5, 6)  # ceil(n*300/256) 256-row K-tiles per branch
KTOT8 = 6  # distinct Xrep K-tiles (256 rows) per sample-group
KROWS = 256 * KTOT8  # 1536 (1500 real + 36 zero rows)
NWARM = 9  # PE clock-gate warmup matmuls (N=512), timed to the first x landing

TRACE = False
LAST_RESULT = None

_built = None


def _build_bass():
    import concourse.mybir as mybir
    import concourse.tile as tile
    from concourse import bacc
    from contextlib import ExitStack

    f32 = mybir.dt.float32
    f8 = mybir.dt.float8e4
    DR = mybir.MatmulPerfMode.DoubleRow

    nc = bacc.Bacc("TRN2", target_bir_lowering=False)
    # [p, sg*6+r, (sample-in-group, i), t] so merged r-groups are contiguous
    xt_d = nc.dram_tensor(
        "xt", (128, 2 * KTOT8, 8, SEQP), f8, kind="ExternalInput"
    )
    # both depth-halves of one branch in a single tensor (col = dh*nt + r)
    w_d = {
        br: nc.dram_tensor(
            f"wb{br}", (128, 2 * NT8[br], 2, 128), f8, kind="ExternalInput"
        )
        for br in range(3)
    }
    # sg-major so each sample-group's slice is one contiguous DMA
    out_d = nc.dram_tensor("out_t", (128, 2, 3, 2, 4), f32, kind="ExternalOutput")

    with tile.TileContext(nc) as tc, ExitStack() as ctx:
        xpool = ctx.enter_context(tc.tile_pool(name="x", bufs=1))
        wpool = ctx.enter_context(tc.tile_pool(name="w", bufs=1))
        cpool = ctx.enter_context(tc.tile_pool(name="consts", bufs=1))
        spool = ctx.enter_context(tc.tile_pool(name="stage", bufs=1))
        pspool = ctx.enter_context(tc.tile_pool(name="ps", bufs=8, space="PSUM"))

        # PE clock-gate warmup operand.  Memset on gpsimd (the earliest-
        # dispatching engine) right after w1's SWDGE descgen, so warmup
        # matmuls start ~8.7us.  HAM ungates at warmup_start+4.9us; real
        # matmuls may begin throttled for <=1us, which costs less than
        # idling behind extra warmups.
        wu = cpool.tile([128, 2, 512], f8)

        # DMA model (measured): per-queue completion posts fire ~1.3us after
        # the transfer's data while the queue is otherwise quiet, but on a
        # still-streaming queue they pace at ~4.2us intervals (receipt
        # processing starves behind data traffic).  Only each queue's first
        # two transfers post promptly.  The sync queue starts ~8.2us; the
        # scalar queue starts ~9.0us if its first transfer reads xt (and
        # ~11.2us if it reads weights first).  The six early-deadline items
        # (w0, r0, r1, r2, r3, w1) are therefore merged into four transfers
        # occupying the four prompt slots; everything later tolerates the
        # 4.2us pacing with >=1us margin.
        gA1 = xpool.tile([128, 2, 8, SEQP], f8, name="gA1")  # sg0 r0,r1
        gR2 = xpool.tile([128, 1, 8, SEQP], f8, name="gR2")  # sg0 r2
        gR3 = xpool.tile([128, 1, 8, SEQP], f8, name="gR3")  # sg0 r3
        gB3 = xpool.tile([128, 2, 8, SEQP], f8, name="gB3")  # sg0 r4,r5
        gA4 = xpool.tile([128, 3, 8, SEQP], f8, name="gA4")  # sg1 r0-r2
        gB4 = xpool.tile([128, 3, 8, SEQP], f8, name="gB4")  # sg1 r3-r5
        wts = {
            br: wpool.tile([128, 2 * NT8[br], 2, 128], f8, name=f"wb{br}")
            for br in range(3)
        }

        # Warmup memset first on gpsimd (warmup start ~8.1us), then the two
        # SWDGE weight loads (measured ~139GB/s; w1 lands ~17.7us, inside
        # its ~20us deadline).
        nc.gpsimd.memset(wu[:], 0)
        nc.gpsimd.dma_start(wts[1][:], w_d[1][:])
        nc.gpsimd.dma_start(wts[2][:], w_d[2][:])

        # sync carries only the two small critical transfers and drains by
        # ~13.5us; scalar carries the rest, with ALL bulk held behind a
        # tiny gate DMA whose trigger waits on gR3's completion semaphore:
        # queue FIFO then keeps the bulk off the wire until scalar's own
        # critical receipts (gA1 ranges, gR3) have posted, and sync's
        # receipts never see competing traffic at all.
        nc.sync.dma_start(wts[0][:], w_d[0][:])
        nc.scalar.dma_start(gA1[:], xt_d[:, 0:2])
        nc.sync.dma_start(gR2[:], xt_d[:, 2:3])
        nc.scalar.dma_start(gR3[:], xt_d[:, 3:4])
        # The gate reads gR3's LAST bytes so a range-tracked wait covers the
        # transfer's final descriptor, not just its first.
        gate = cpool.tile([128, 16], f8)
        nc.scalar.dma_start(gate[:], gR3[:, 0, 7, SEQP - 16 : SEQP])
        nc.scalar.dma_start(gB3[:], xt_d[:, 4:6])
        nc.scalar.dma_start(gA4[:], xt_d[:, 6:9])
        nc.scalar.dma_start(gB4[:], xt_d[:, 9:12])

        def xap(sg, r):
            if sg == 0:
                g, i = (
                    (gA1, r)
                    if r < 2
                    else (gR2, 0)
                    if r == 2
                    else (gR3, 0)
                    if r == 3
                    else (gB3, r - 4)
                )
            else:
                g, i = (gA4, r) if r < 3 else (gB4, r - 3)
            return g[:, i]

        # Warm the HAM clock gate while the first DMAs land (PE would
        # otherwise sit idle and start cold at 1.2GHz).
        ps_wu = pspool.tile([128, 512], f32, tag="ps", name="ps_wu")
        for k in range(NWARM):
            nc.tensor.matmul(
                ps_wu[:, :512],
                lhsT=wu[:, :, :128],
                rhs=wu[:],
                start=True,
                stop=True,
                perf_mode=DR,
            )

        stage2 = spool.tile([128, 2, 3, 2, 4], f32)

        for sg in range(2):
            for br in range(3):
                nt = NT8[br]
                nw = SEQ - NS[br]  # windows the reference maxes over
                pss = {
                    (dh, j): pspool.tile(
                        [128, 512], f32, tag="ps", name=f"ps_{br}_{sg}_{dh}_{j}"
                    )
                    for dh in range(2)
                    for j in range(4)
                }
                for r in range(nt):
                    xt_r = xap(sg, r)
                    for dh in range(2):
                        for j in range(4):
                            nc.tensor.matmul(
                                pss[dh, j][:, :nw],
                                lhsT=wts[br][:, dh * nt + r, :, :],
                                rhs=xt_r[:, 2 * j : 2 * j + 2, :nw],
                                start=(r == 0),
                                stop=(r == nt - 1),
                                perf_mode=DR,
                            )
                for dh in range(2):
                    for j in range(4):
                        nc.vector.reduce_max(
                            stage2[:, sg, br, dh, j : j + 1],
                            pss[dh, j][:, :nw],
                            axis=mybir.AxisListType.X,
                        )
            # Bias + relu happen on the host (kernel() epilogue): the device
            # tail is just the last reduce plus this DMA.  sg0's transfer and
            # sg1's first two branches hide under the matmul stream; only
            # branch 2 of sg1 (8KB) rides the critical tail.
            if sg == 0:
                nc.scalar.dma_start(out_d[:, 0], stage2[:, 0])
            else:
                nc.scalar.dma_start(out_d[:, 1, 0:2], stage2[:, 1, 0:2])
                nc.scalar.dma_start(out_d[:, 1, 2:3], stage2[:, 1, 2:3])

    nc.compile()
    return nc


def _pack_inputs(input, W1, W2, W3, b1, b2, b3):
    f8 = ml_dtypes.float8_e4m3

    # Host-materialized im2col: Xrep[b, k, t] = x[b, t + k//300, k%300],
    # SEQ padded to 400 with zeros, K padded to 1536 with zero rows.
    xt = np.zeros((B, EMB, SEQP), np.float32)
    xt[:, :, :SEQ] = np.asarray(input, np.float32).transpose(0, 2, 1)
    xrep = np.zeros((B, KROWS, SEQP), np.float32)
    for j in range(5):
        xrep[:, j * EMB : (j + 1) * EMB, : SEQP - j] = xt[:, :, j:]
    # global row c = 256r + 128i + p  ->  [b, r, p, i, t]
    x8 = (
        xrep.reshape(B, KTOT8, 2, 128, SEQP)
        .transpose(0, 1, 3, 2, 4)
        .astype(f8)
    )  # [B, 6, 128, 2, 400]

    ws = {}
    for br, (n, W) in enumerate(zip(NS, (W1, W2, W3))):
        Wp = np.zeros((KROWS, DEPTH), np.float32)
        Wp[: n * EMB] = np.asarray(W, np.float32).T
        v = Wp.reshape(KTOT8, 2, 128, 2, 128)  # (r, i, p, dh, m)
        halves = [
            v[: NT8[br], :, :, dh, :].transpose(2, 0, 1, 3)  # (p, r, i, m)
            for dh in range(2)
        ]
        ws[br] = np.ascontiguousarray(np.concatenate(halves, axis=1)).astype(
            f8
        )  # (p, dh*nt+r, i, m)

    return x8, ws


def kernel(input, W1, W2, W3, b1, b2, b3):
    global _built, LAST_RESULT
    from concourse.bass_utils import run_bass_kernel_spmd

    x8, ws = _pack_inputs(input, W1, W2, W3, b1, b2, b3)

    if _built is None:
        _built = _build_bass()
    nc = _built

    in_maps = []
    for c in range(NCORES):
        cx = x8[c * BPC : (c + 1) * BPC]  # [8, 6, 128, 2, 400] (s, r, p, i, t)
        # -> [p, sg*6+r, (s-in-group, i), t]
        xt = np.concatenate(
            [
                cx[sg * 4 : (sg + 1) * 4]
                .transpose(1, 2, 0, 3, 4)  # (r, p, s4, i, t)
                .reshape(KTOT8, 128, 8, SEQP)
                for sg in range(2)
            ]
        ).transpose(1, 0, 2, 3)  # (p, sg*6+r, s4i, t)
        m = {"xt": np.ascontiguousarray(xt)}
        for br in range(3):
            m[f"wb{br}"] = ws[br]
        in_maps.append(m)

    res = run_bass_kernel_spmd(
        nc, in_maps, core_ids=list(range(NCORES)), trace=TRACE
    )
    LAST_RESULT = res

    # Host epilogue: relu(max + b) == max(0, max + b); the device returns the
    # raw per-(depth, branch) window maxima.
    bvec = np.concatenate(
        [np.asarray(b, np.float32).reshape(-1) for b in (b1, b2, b3)]
    )
    out = np.empty((B, 3 * DEPTH), np.float32)
    for c in range(NCORES):
        arr = res.results[c]["out_t"]  # [128, 2, 3, 2, 4] (p, sg, br, dh, s4)
        out[c * BPC : (c + 1) * BPC] = arr.transpose(1, 4, 2, 3, 0).reshape(
            BPC, 768
        )
    np.maximum(out + bvec, 0.0, out=out)
    return out


# revision 49
# speedup vs baseline: 1.0087x; 1.0087x over previous
"""TextCNN-style conv layer (kernel sizes 3/4/5, EMB=300 -> DEPTH=256, bias,
ReLU, max-pool over time) as a Bass/Tile kernel for 8 Trainium2 NeuronCores.

Strategy: data-parallel over batch (8 samples per core), weights replicated.

Conv as dense-K matmuls over a host-materialized im2col matrix
Xrep[k, t] = x[t + k//300, k%300], shared by all three branches (branch n
reads rows [0, n*300), its weights zero-padded to the K-tile boundary).

fp8 e4m3 + DoubleRow: the PE virtualizes to 128x256, contracting 256 rows
per matmul (2 fp8 weights per cell), so each branch needs ceil(n*300/256)
K-tiles: 4/5/6 -> 15 matmuls per sample per depth-half vs 30 at K=128.
Both operands quantize to e4m3; measured end-to-end L2 error vs the fp32
reference is ~1.2e-2 (accumulation stays fp32 in PSUM).

Schedule: sample-group-of-4 OUTER, then branch, then r, then (half,
sample) inner: the 8 concurrent accumulations (2 halves x 4 samples)
exactly fill the 8 PSUM banks, each weight tile serves 4 consecutive
matmuls (LDWEIGHTS hides under the matmul stream), and each x tile is
consumed by both depth-halves AND all three branches before the next
sample-group needs fresh data.  The resulting 240-matmul stream runs
gapless at the fp8 PE roofline (~39.8us at 2.4GHz).

DMA model (measured on HW): a queue's completion post fires ~1.3us
after its transfer's data when the queue is quiet, but on a
still-streaming queue posts starve behind the data traffic (receipts
can lag 2-5us), and per-queue burst rates vary run-to-run between
~142 and ~243GB/s (shared-HBM arbitration lottery).  The schedule
therefore keeps every queue QUIET behind its critical transfers:
sync carries only w0 + r2 (1.05MB, drains ~13.5us, both receipts
prompt); gpsimd/SWDGE (~139GB/s) carries w1 + w2, whose ~20/26us
deadlines tolerate it; scalar carries [r0+r1] and r3, then a 2KB
gate DMA that reads a slice of r3 -- its trigger waits on r3's
completion semaphore, so queue FIFO holds all bulk ([r4+r5] and the
two 1.2MB sg1 groups) off the wire until scalar's own critical
receipts have posted.  First real matmul gates on w0 (post ~10.8us)
+ the r0 range of [r0+r1] (~12.3-15us); the 308GB/s steady demand
sits under the 358GB/s HBM roofline.

A stream of dummy matmuls on a memset tile warms the PE HAM clock gate
(1.2 -> 2.4GHz after ~4.9us of sustained PE activity).  The memset is
gpsimd's first kernel instruction (earliest-dispatching engine), so
warmup matmuls start ~8.1us; the real stream may begin throttled for
<=1us, which costs less than idling behind extra warmups.

Epilogue: relu(max_i(y + b)) == max(0, max_i y + b): DVE reduce_max
over the window axis straight out of PSUM; bias-add + relu fold into
the host-side unpack (a [64, 768] numpy op), so the device tail is
just the final reduce plus an 8KB DMA.  Output staged
[d, sg, branch, half, sample]: sg0's transfer and sg1's first two
branches hide under the matmul stream.
"""

import numpy as np
import ml_dtypes

B, SEQ, EMB = 64, 394, 300
DEPTH = 256
NCORES = 8
BPC = B // NCORES  # samples per core
SEQP = 400  # x_t free-dim padded (zeros) so shifted loads stay in bounds
NS = (3, 4, 5)
NT8 = (4, 5, 6)  # ceil(n*300/256) 256-row K-tiles per branch
KTOT8 = 6  # distinct Xrep K-tiles (256 rows) per sample-group
KROWS = 256 * KTOT8  # 1536 (1500 real + 36 zero rows)
NWARM = 9  # PE clock-gate warmup matmuls (N=512), timed to the first x landing

TRACE = False
LAST_RESULT = None

_built = None


def _build_bass():
    import concourse.mybir as mybir
    import concourse.tile as tile
    from concourse import bacc
    from contextlib import ExitStack

    f32 = mybir.dt.float32
    f8 = mybir.dt.float8e4
    DR = mybir.MatmulPerfMode.DoubleRow

    nc = bacc.Bacc("TRN2", target_bir_lowering=False)
    # [p, sg*6+r, (sample-in-group, i), t] so merged r-groups are contiguous
    xt_d = nc.dram_tensor(
        "xt", (128, 2 * KTOT8, 8, SEQP), f8, kind="ExternalInput"
    )
    # both depth-halves of one branch in a single tensor (col = dh*nt + r)
    w_d = {
        br: nc.dram_tensor(
            f"wb{br}", (128, 2 * NT8[br], 2, 128), f8, kind="ExternalInput"
        )
        for br in range(3)
    }
    # sg-major so each sample-group's slice is one contiguous DMA
    out_d = nc.dram_tensor("out_t", (128, 2, 3, 2, 4), f32, kind="ExternalOutput")

    with tile.TileContext(nc) as tc, ExitStack() as ctx:
        xpool = ctx.enter_context(tc.tile_pool(name="x", bufs=1))
        wpool = ctx.enter_context(tc.tile_pool(name="w", bufs=1))
        cpool = ctx.enter_context(tc.tile_pool(name="consts", bufs=1))
        spool = ctx.enter_context(tc.tile_pool(name="stage", bufs=1))
        pspool = ctx.enter_context(tc.tile_pool(name="ps", bufs=8, space="PSUM"))

        # PE clock-gate warmup operand.  Memset on gpsimd (the earliest-
        # dispatching engine) right after w1's SWDGE descgen, so warmup
        # matmuls start ~8.7us.  HAM ungates at warmup_start+4.9us; real
        # matmuls may begin throttled for <=1us, which costs less than
        # idling behind extra warmups.
        wu = cpool.tile([128, 2, 512], f8)

        # DMA model (measured): per-queue completion posts fire ~1.3us after
        # the transfer's data while the queue is otherwise quiet, but on a
        # still-streaming queue they pace at ~4.2us intervals (receipt
        # processing starves behind data traffic).  Only each queue's first
        # two transfers post promptly.  The sync queue starts ~8.2us; the
        # scalar queue starts ~9.0us if its first transfer reads xt (and
        # ~11.2us if it reads weights first).  The six early-deadline items
        # (w0, r0, r1, r2, r3, w1) are therefore merged into four transfers
        # occupying the four prompt slots; everything later tolerates the
        # 4.2us pacing with >=1us margin.
        gA1 = xpool.tile([128, 2, 8, SEQP], f8, name="gA1")  # sg0 r0,r1
        gR2 = xpool.tile([128, 1, 8, SEQP], f8, name="gR2")  # sg0 r2
        gR3 = xpool.tile([128, 1, 8, SEQP], f8, name="gR3")  # sg0 r3
        gB3 = xpool.tile([128, 2, 8, SEQP], f8, name="gB3")  # sg0 r4,r5
        gA4 = xpool.tile([128, 3, 8, SEQP], f8, name="gA4")  # sg1 r0-r2
        gB4 = xpool.tile([128, 3, 8, SEQP], f8, name="gB4")  # sg1 r3-r5
        wts = {
            br: wpool.tile([128, 2 * NT8[br], 2, 128], f8, name=f"wb{br}")
            for br in range(3)
        }

        # Warmup memset first on gpsimd (warmup start ~8.1us), then the two
        # SWDGE weight loads (measured ~139GB/s; w1 lands ~17.7us, inside
        # its ~20us deadline).
        nc.gpsimd.memset(wu[:], 0)
        nc.gpsimd.dma_start(wts[1][:], w_d[1][:])
        nc.gpsimd.dma_start(wts[2][:], w_d[2][:])

        # sync carries only the two small critical transfers and drains by
        # ~13.5us; scalar carries the rest, with ALL bulk held behind a
        # tiny gate DMA whose trigger waits on gR3's completion semaphore:
        # queue FIFO then keeps the bulk off the wire until scalar's own
        # critical receipts (gA1 ranges, gR3) have posted, and sync's
        # receipts never see competing traffic at all.
        nc.sync.dma_start(wts[0][:], w_d[0][:])
        nc.scalar.dma_start(gA1[:], xt_d[:, 0:2])
        nc.sync.dma_start(gR2[:], xt_d[:, 2:3])
        nc.scalar.dma_start(gR3[:], xt_d[:, 3:4])
        gate = cpool.tile([128, 16], f8)
        nc.scalar.dma_start(gate[:], gR3[:, 0, 0, 0:16])
        nc.scalar.dma_start(gB3[:], xt_d[:, 4:6])
        nc.scalar.dma_start(gA4[:], xt_d[:, 6:9])
        nc.scalar.dma_start(gB4[:], xt_d[:, 9:12])

        def xap(sg, r):
            if sg == 0:
                g, i = (
                    (gA1, r)
                    if r < 2
                    else (gR2, 0)
                    if r == 2
                    else (gR3, 0)
                    if r == 3
                    else (gB3, r - 4)
                )
            else:
                g, i = (gA4, r) if r < 3 else (gB4, r - 3)
            return g[:, i]

        # Warm the HAM clock gate while the first DMAs land (PE would
        # otherwise sit idle and start cold at 1.2GHz).
        ps_wu = pspool.tile([128, 512], f32, tag="ps", name="ps_wu")
        for k in range(NWARM):
            nc.tensor.matmul(
                ps_wu[:, :512],
                lhsT=wu[:, :, :128],
                rhs=wu[:],
                start=True,
                stop=True,
                perf_mode=DR,
            )

        stage2 = spool.tile([128, 2, 3, 2, 4], f32)

        for sg in range(2):
            for br in range(3):
                nt = NT8[br]
                nw = SEQ - NS[br]  # windows the reference maxes over
                pss = {
                    (dh, j): pspool.tile(
                        [128, 512], f32, tag="ps", name=f"ps_{br}_{sg}_{dh}_{j}"
                    )
                    for dh in range(2)
                    for j in range(4)
                }
                for r in range(nt):
                    xt_r = xap(sg, r)
                    for dh in range(2):
                        for j in range(4):
                            nc.tensor.matmul(
                                pss[dh, j][:, :nw],
                                lhsT=wts[br][:, dh * nt + r, :, :],
                                rhs=xt_r[:, 2 * j : 2 * j + 2, :nw],
                                start=(r == 0),
                                stop=(r == nt - 1),
                                perf_mode=DR,
                            )
                for dh in range(2):
                    for j in range(4):
                        nc.vector.reduce_max(
                            stage2[:, sg, br, dh, j : j + 1],
                            pss[dh, j][:, :nw],
                            axis=mybir.AxisListType.X,
                        )
            # Bias + relu happen on the host (kernel() epilogue): the device
            # tail is just the last reduce plus this DMA.  sg0's transfer and
            # sg1's first two branches hide under the matmul stream; only
            # branch 2 of sg1 (8KB) rides the critical tail.
            if sg == 0:
                nc.scalar.dma_start(out_d[:, 0], stage2[:, 0])
            else:
                nc.scalar.dma_start(out_d[:, 1, 0:2], stage2[:, 1, 0:2])
                nc.scalar.dma_start(out_d[:, 1, 2:3], stage2[:, 1, 2:3])

    nc.compile()
    return nc


def _pack_inputs(input, W1, W2, W3, b1, b2, b3):
    f8 = ml_dtypes.float8_e4m3

    # Host-materialized im2col: Xrep[b, k, t] = x[b, t + k//300, k%300],
    # SEQ padded to 400 with zeros, K padded to 1536 with zero rows.
    xt = np.zeros((B, EMB, SEQP), np.float32)
    xt[:, :, :SEQ] = np.asarray(input, np.float32).transpose(0, 2, 1)
    xrep = np.zeros((B, KROWS, SEQP), np.float32)
    for j in range(5):
        xrep[:, j * EMB : (j + 1) * EMB, : SEQP - j] = xt[:, :, j:]
    # global row c = 256r + 128i + p  ->  [b, r, p, i, t]
    x8 = (
        xrep.reshape(B, KTOT8, 2, 128, SEQP)
        .transpose(0, 1, 3, 2, 4)
        .astype(f8)
    )  # [B, 6, 128, 2, 400]

    ws = {}
    for br, (n, W) in enumerate(zip(NS, (W1, W2, W3))):
        Wp = np.zeros((KROWS, DEPTH), np.float32)
        Wp[: n * EMB] = np.asarray(W, np.float32).T
        v = Wp.reshape(KTOT8, 2, 128, 2, 128)  # (r, i, p, dh, m)
        halves = [
            v[: NT8[br], :, :, dh, :].transpose(2, 0, 1, 3)  # (p, r, i, m)
            for dh in range(2)
        ]
        ws[br] = np.ascontiguousarray(np.concatenate(halves, axis=1)).astype(
            f8
        )  # (p, dh*nt+r, i, m)

    return x8, ws


def kernel(input, W1, W2, W3, b1, b2, b3):
    global _built, LAST_RESULT
    from concourse.bass_utils import run_bass_kernel_spmd

    x8, ws = _pack_inputs(input, W1, W2, W3, b1, b2, b3)

    if _built is None:
        _built = _build_bass()
    nc = _built

    in_maps = []
    for c in range(NCORES):
        cx = x8[c * BPC : (c + 1) * BPC]  # [8, 6, 128, 2, 400] (s, r, p, i, t)
        # -> [p, sg*6+r, (s-in-group, i), t]
        xt = np.concatenate(
            [
                cx[sg * 4 : (sg + 1) * 4]
                .transpose(1, 2, 0, 3, 4)  # (r, p, s4, i, t)
                .reshape(KTOT8, 128, 8, SEQP)
                for sg in range(2)
            ]
        ).transpose(1, 0, 2, 3)  # (p, sg*6+r, s4i, t)
        m = {"xt": np.ascontiguousarray(xt)}
        for br in range(3):
            m[f"wb{br}"] = ws[br]
        in_maps.append(m)

    res = run_bass_kernel_spmd(
        nc, in_maps, core_ids=list(range(NCORES)), trace=TRACE
    )
    LAST_RESULT = res

    # Host epilogue: relu(max + b) == max(0, max + b); the device returns the
    # raw per-(depth, branch) window maxima.
    bvec = np.concatenate(
        [np.asarray(b, np.float32).reshape(-1) for b in (b1, b2, b3)]
    )
    out = np.empty((B, 3 * DEPTH), np.float32)
    for c in range(NCORES):
        arr = res.results[c]["out_t"]  # [128, 2, 3, 2, 4] (p, sg, br, dh, s4)
        out[c * BPC : (c + 1) * BPC] = arr.transpose(1, 4, 2, 3, 0).reshape(
            BPC, 768
        )
    np.maximum(out + bvec, 0.0, out=out)
    return out
